# revision 13
# baseline (speedup 1.0000x reference)
"""AttentionAggregation Trainium2 kernel (8-core SPMD).

Math (per batch b):
    t_i   = w . (mask_b * hidden[b, i])         for i in [start_b, end_b)
    e_i   = exp(t_i)
    num   = sum_i e_i * hidden[b, i]
    den   = sum_i e_i + (max_len - qlen_b)      # padded zero-logit positions
    out_b = mask_b * num / den

Device does all O(rows * H) work; the host slices the contiguous
[start, end) row ranges into 128-row tiles, distributes fixed-size tile
chains across the 8 cores, and runs an O(B*H) epilogue combining
per-chain partial sums.

Per-core device kernel (value-agnostic; all data dependence flows through
input tensors). For each 128-row tile k (chain ch = k // G):
    t[:,k]  = sum_free((hs_k * mexp[:,k]) * w_rep)   VectorE fused
              scalar_tensor_tensor with accum_out
    e[:,k]  = exp(t[:,k] + bias[:,k])                ScalarE (bias=-1e5
              marks invalid rows -> e exactly 0)
    psum[slot(ch)] += e[:,k].T @ hs_k                TensorE, 2 fp32
                                                     N=512 matmuls
PSUM slot s = 2*ch+h lives at (bank s//4, partition 32*(s%4)); banks are
evacuated (ScalarE copy + DMA) as soon as their two chains finish.
"""

import os
from contextlib import ExitStack

import numpy as np

import concourse.bacc as bacc
import concourse.bass as bass
import concourse.mybir as mybir
from concourse import tile
from concourse.bass_utils import run_bass_kernel_spmd

F32 = mybir.dt.float32
P = 128          # SBUF partitions / rows per tile
NCORES = 8
DG = 4           # tiles per DMA group (2 MiB transfers)
MAX_NCH = 128    # chains per core (PSUM pair tiles are recycled)
NEG_BIAS = -1.0e5

_NC_CACHE: dict = {}


def _build_nc(T: int, G: int, NCH: int, H: int, NGP: int = 0):
    nc = bacc.Bacc("TRN2", target_bir_lowering=False, debug=False)
    hs = nc.dram_tensor("hs", [T, P, H], F32, kind="ExternalInput")
    wrep = nc.dram_tensor("wrep", [P, H], F32, kind="ExternalInput")
    aux = nc.dram_tensor("aux", [P, 2 * T], F32, kind="ExternalInput")
    eout = nc.dram_tensor("eout", [P, T], F32, kind="ExternalOutput")
    nout = nc.dram_tensor("nout", [2 * NCH, 512], F32, kind="ExternalOutput")

    nslots = 2 * NCH
    H2 = H // 2
    # tiles whose multiply runs on GpSimd (reduce then on ScalarE), to
    # offload the VectorE fused op. Spread through the middle.
    gp_tiles = set()
    if NGP:
        step = max(1, T // NGP)
        gp_tiles = {3 + i * step for i in range(NGP) if 3 + i * step < T - 1}

    # DMA groups: single-tile first groups so compute starts early
    groups = []
    k = 0
    for n in [1, 1, 2]:
        if k + n > T:
            break
        groups.append((k, n))
        k += n
    while k < T:
        n = min(DG, T - k)
        groups.append((k, n))
        k += n

    with tile.TileContext(nc) as tc, ExitStack() as ctx:
        cpool = ctx.enter_context(tc.tile_pool(name="consts", bufs=1))
        sb = ctx.enter_context(tc.tile_pool(name="hsbuf", bufs=6))
        scr = ctx.enter_context(tc.tile_pool(name="scr", bufs=2))
        evacp = ctx.enter_context(tc.tile_pool(name="evac", bufs=1))
        perm = ctx.enter_context(tc.tile_pool(name="perm", bufs=1))
        psum = ctx.enter_context(
            tc.tile_pool(name="ps", bufs=8, space=bass.MemorySpace.PSUM)
        )

        aux_sb = cpool.tile([P, 2 * T], F32)
        nc.sync.dma_start(aux_sb[:], aux[:])
        wrep_sb = cpool.tile([P, H], F32)
        nc.sync.dma_start(wrep_sb[:], wrep[:])

        t_all = perm.tile([P, T], F32)
        e_all = perm.tile([P, T], F32)
        dumm = perm.tile([P, 1], F32)

        npairs = (NCH + 1) // 2
        pair_tile = [None]
        evac_all = evacp.tile([P, npairs, 512], F32)

        def evac_pair(p):
            nc.scalar.copy(evac_all[:, p, :], pair_tile[0][:])

        for g0, gn in groups:
            gt = sb.tile([P, DG, H], F32, tag="gt")
            src = hs[g0 : g0 + gn].rearrange("j p h -> p j h")
            nc.sync.dma_start(gt[:, 0:gn, :], src)
            for j in range(gn):
                k = g0 + j
                ch = k // G
                if k in gp_tiles:
                    # multiply on GpSimd, mask+reduce on ScalarE
                    s = scr.tile([P, H], F32, tag="s")
                    nc.gpsimd.tensor_tensor(
                        s[:], gt[:, j, :], wrep_sb[:], op=mybir.AluOpType.mult
                    )
                    nc.scalar.activation(
                        dumm.broadcast_to(s[:].shape),
                        s[:],
                        mybir.ActivationFunctionType.Copy,
                        scale=aux_sb[:, k : k + 1],
                        accum_out=t_all[:, k : k + 1],
                    )
                else:
                    # dot + mask + reduce fused on VectorE
                    nc.vector.scalar_tensor_tensor(
                        dumm.broadcast_to(gt[:, j, :].shape),
                        gt[:, j, :],
                        aux_sb[:, k : k + 1],
                        wrep_sb[:],
                        op0=mybir.AluOpType.mult,
                        op1=mybir.AluOpType.mult,
                        accum_out=t_all[:, k : k + 1],
                    )
                nc.scalar.activation(
                    e_all[:, k : k + 1],
                    t_all[:, k : k + 1],
                    mybir.ActivationFunctionType.Exp,
                    bias=aux_sb[:, T + k : T + k + 1],
                )
                if k % (2 * G) == 0:
                    pair_tile[0] = psum.tile([P, 512], F32, name=f"pair{k}", tag="pair")
                for h in range(2):
                    slot = 2 * ch + h
                    part = 32 * (slot % 4)
                    nc.tensor.matmul(
                        pair_tile[0][part : part + 1, :],
                        lhsT=e_all[:, k : k + 1],
                        rhs=gt[:, j, h * H2 : (h + 1) * H2],
                        start=(k % G == 0),
                        stop=(k % G == G - 1),
                        tile_position=(0, part),
                    )
                # pair complete once its odd chain (or the last chain) ends
                if k % G == G - 1 and (ch % 2 == 1 or ch == NCH - 1):
                    evac_pair(ch // 2)
        # nout[4p+q] = evac_all[32q, p, :]
        nparts = min(4, nslots)
        nout_v = nout.rearrange("(p q) n -> q p n", q=nparts)
        nc.sync.dma_start(nout_v, evac_all[0 : 32 * nparts : 32, :, :])
        nc.sync.dma_start(eout[:], e_all[:])
    nc.finalize()
    return nc


def _choose_layout(tiles_per_batch):
    """Pick chain granularity G and per-core chain count NCH.

    PSUM banks are recycled (pool of 8 rotating pair tiles), so NCH is
    only limited by keeping the nout output reasonable."""
    best = None
    for G in (1, 2, 4, 8, 16, 32, 64):
        chains_total = sum((tb + G - 1) // G for tb in tiles_per_batch)
        nch = max(1, (chains_total + NCORES - 1) // NCORES)
        if nch > MAX_NCH:
            continue
        T = nch * G
        if best is None or T < best[2]:
            best = (G, nch, T)
    if best is None:
        raise ValueError(f"no feasible layout for {tiles_per_batch}")
    return best


def kernel(hidden_states, query_indices, attention_mask, w):
    hidden_states = np.asarray(hidden_states, dtype=np.float32)
    qi = np.asarray(query_indices)
    mask = np.asarray(attention_mask, dtype=np.float32)
    w = np.asarray(w, dtype=np.float32)
    B, S, H = hidden_states.shape

    start = qi[:, 0].astype(np.int64)
    end = qi[:, 1].astype(np.int64)
    qlen = end - start
    max_len = int((end + 1 - start).max())

    mode = os.environ.get("ATTNAGG_MODE", "gather")
    if mode == "full":
        tiles_per_batch = [(S + P - 1) // P] * B
    else:
        tiles_per_batch = [int((q + P - 1) // P) for q in qlen]

    G, NCH, T = _choose_layout(tiles_per_batch)

    # ---- global chain list: (batch, first_tile_in_batch, ntiles) ----
    chains = []
    for b in range(B):
        tb = tiles_per_batch[b]
        for c in range((tb + G - 1) // G):
            chains.append((b, c * G, min(G, tb - c * G)))
    assert len(chains) <= NCORES * NCH

    # ---- pack per-core inputs ----
    wrep = np.ascontiguousarray(np.broadcast_to(w, (P, H)))
    in_maps = []
    core_chains = [[] for _ in range(NCORES)]  # (slot, batch, first_tile, ntiles)
    for i, (b, ft, nt) in enumerate(chains):
        core_chains[i // NCH].append((i % NCH, b, ft, nt))

    for c in range(NCORES):
        hs_shard = np.zeros((T, P, H), dtype=np.float32)
        aux = np.zeros((P, 2 * T), dtype=np.float32)
        aux[:, T:] = NEG_BIAS
        for slot, b, ft, nt in core_chains[c]:
            for tg in range(nt):
                k = slot * G + tg
                tb_idx = ft + tg
                if mode == "full":
                    r0 = tb_idx * P
                    r1 = min(r0 + P, S)
                    n = r1 - r0
                    hs_shard[k, :n] = hidden_states[b, r0:r1]
                    rows = np.arange(r0, r1)
                    valid = (rows >= start[b]) & (rows < end[b])
                    aux[:n, T + k] = np.where(valid, 0.0, NEG_BIAS)
                else:
                    r0 = int(start[b]) + tb_idx * P
                    r1 = min(r0 + P, int(end[b]))
                    n = r1 - r0
                    hs_shard[k, :n] = hidden_states[b, r0:r1]
                    aux[:n, T + k] = 0.0
                aux[:, k] = mask[b]
        in_maps.append({"hs": hs_shard, "wrep": wrep, "aux": aux})

    # ---- build + run ----
    ngp = int(os.environ.get("ATTNAGG_NGP", "0"))
    key = (T, G, NCH, H, ngp)
    if key not in _NC_CACHE:
        _NC_CACHE[key] = _build_nc(T, G, NCH, H, ngp)
    nc = _NC_CACHE[key]

    trace = bool(int(os.environ.get("ATTNAGG_TRACE", "0")))
    kw = {}
    if trace:
        import concourse.bass_utils as _bu

        _bu.upload_artifacts = lambda tmpdir: "local://" + tmpdir
        tdir = os.environ.get("ATTNAGG_TRACE_DIR") or None
        if tdir:
            import glob as _glob

            for f in _glob.glob(os.path.join(tdir, "*")):
                try:
                    os.remove(f)
                except OSError:
                    pass
        kw = {"trace": True, "tmpdir": tdir}
    res = run_bass_kernel_spmd(nc, in_maps, list(range(NCORES)), **kw)
    if trace:
        kernel.last_exec_time_ns = res.exec_time_ns
        kernel.last_trace = res.instructions_and_trace

    # ---- host epilogue (tiny): combine per-chain partials ----
    num = np.zeros((B, H), dtype=np.float64)
    den = np.zeros((B,), dtype=np.float64)
    for c in range(NCORES):
        eo = res.results[c]["eout"].astype(np.float64)
        no = res.results[c]["nout"].astype(np.float64)
        for slot, b, ft, nt in core_chains[c]:
            num[b, :512] += no[2 * slot]
            num[b, 512:] += no[2 * slot + 1]
            den[b] += eo[:, slot * G : slot * G + nt].sum()
    den += (max_len - qlen).astype(np.float64)
    out = mask.astype(np.float64)[:, None] * num / den[:, None]
    return out.astype(np.float32)


# revision 23
# speedup vs baseline: 1.1955x; 1.1955x over previous
"""AttentionAggregation Trainium2 kernel (8-core SPMD).

Math (per batch b):
    t_i   = w . (mask_b * hidden[b, i])         for i in [start_b, end_b)
    e_i   = exp(t_i)
    num   = sum_i e_i * hidden[b, i]
    den   = sum_i e_i + (max_len - qlen_b)      # padded zero-logit positions
    out_b = mask_b * num / den

Device does all O(rows * H) work; the host slices the contiguous
[start, end) row ranges into 128-row tiles, distributes fixed-size tile
chains across the 8 cores, and runs an O(B*H) epilogue combining
per-chain partial sums.

Per-core device kernel (value-agnostic; all data dependence flows through
input tensors). For each 128-row tile k (chain ch = k // G):
    t[:,k]  = sum_free((hs_k * mexp[:,k]) * w_rep)   VectorE fused
              scalar_tensor_tensor with accum_out
    e[:,k]  = exp(t[:,k] + bias[:,k])                ScalarE (bias=-1e5
              marks invalid rows -> e exactly 0)
    psum[slot(ch)] += e[:,k].T @ hs_k                TensorE, 2 fp32
                                                     N=512 matmuls
Each pair of chains owns one PSUM bank tile (from a rotating pool of
8, so the chain count is not limited by the 8 physical banks); slot
s = 2*ch+h sits at partition 32*(s%4) of the pair's bank. A bank is
evacuated (ScalarE copy + DMA to nout) as soon as its two chains finish,
freeing it for a later pair.
"""

import os
from contextlib import ExitStack

import numpy as np

import concourse.bacc as bacc
import concourse.bass as bass
import concourse.mybir as mybir
from concourse import tile
from concourse.bass_utils import run_bass_kernel_spmd

F32 = mybir.dt.float32
P = 128          # SBUF partitions / rows per tile
NCORES = 8
DG = 4           # tiles per DMA group (2 MiB transfers)
MAX_NCH = 128    # chains per core (PSUM pair tiles are recycled)
NEG_BIAS = -1.0e5

_NC_CACHE: dict = {}


def _build_nc(T: int, G: int, NCH: int, H: int):
    nc = bacc.Bacc("TRN2", target_bir_lowering=False, debug=False)
    hs = nc.dram_tensor("hs", [T, P, H], F32, kind="ExternalInput")
    wrep = nc.dram_tensor("wrep", [P, H], F32, kind="ExternalInput")
    aux = nc.dram_tensor("aux", [P, 2 * T], F32, kind="ExternalInput")
    eout = nc.dram_tensor("eout", [P, T], F32, kind="ExternalOutput")
    nout = nc.dram_tensor("nout", [2 * NCH, 512], F32, kind="ExternalOutput")

    nslots = 2 * NCH
    H2 = H // 2

    # DMA groups: single-tile first groups so compute starts early
    groups = []
    k = 0
    for n in [1, 1, 1, 1]:
        if k + n > T:
            break
        groups.append((k, n))
        k += n
    while k < T:
        n = min(DG, T - k)
        groups.append((k, n))
        k += n

    with tile.TileContext(nc) as tc, ExitStack() as ctx:
        cpool = ctx.enter_context(tc.tile_pool(name="consts", bufs=1))
        sb = ctx.enter_context(tc.tile_pool(name="hsbuf", bufs=8))
        evacp = ctx.enter_context(tc.tile_pool(name="evac", bufs=1))
        perm = ctx.enter_context(tc.tile_pool(name="perm", bufs=1))
        psum = ctx.enter_context(
            tc.tile_pool(name="ps", bufs=8, space=bass.MemorySpace.PSUM)
        )

        aux_sb = cpool.tile([P, 2 * T], F32)
        nc.sync.dma_start(aux_sb[:], aux[:])
        wrep_sb = cpool.tile([P, H], F32)
        nc.sync.dma_start(wrep_sb[:], wrep[:])

        t_all = perm.tile([P, T], F32)
        e_all = perm.tile([P, T], F32)
        dumm = perm.tile([P, 1], F32)

        npairs = (NCH + 1) // 2
        pair_tile = [None]

        def evac_pair(p):
            lo = 4 * p
            hi = min(4 * p + 4, nslots)
            ev = evacp.tile([P, 512], F32, tag="ev", bufs=2)
            nc.scalar.copy(ev[:], pair_tile[0][:])
            nc.sync.dma_start(nout[lo:hi, :], ev[0 : 32 * (hi - lo) : 32, :])

        for g0, gn in groups:
            gt = sb.tile([P, DG, H], F32, tag="gt")
            src = hs[g0 : g0 + gn].rearrange("j p h -> p j h")
            nc.sync.dma_start(gt[:, 0:gn, :], src)
            for j in range(gn):
                k = g0 + j
                ch = k // G
                # dot + mask + reduce fused on VectorE:
                # t[:,k] = sum_h (hs * mask_b) * w
                nc.vector.scalar_tensor_tensor(
                    dumm.broadcast_to(gt[:, j, :].shape),
                    gt[:, j, :],
                    aux_sb[:, k : k + 1],
                    wrep_sb[:],
                    op0=mybir.AluOpType.mult,
                    op1=mybir.AluOpType.mult,
                    accum_out=t_all[:, k : k + 1],
                )
                nc.scalar.activation(
                    e_all[:, k : k + 1],
                    t_all[:, k : k + 1],
                    mybir.ActivationFunctionType.Exp,
                    bias=aux_sb[:, T + k : T + k + 1],
                )
                if k % (2 * G) == 0:
                    pair_tile[0] = psum.tile([P, 512], F32, name=f"pair{k}", tag="pair")
                for h in range(2):
                    slot = 2 * ch + h
                    part = 32 * (slot % 4)
                    nc.tensor.matmul(
                        pair_tile[0][part : part + 1, :],
                        lhsT=e_all[:, k : k + 1],
                        rhs=gt[:, j, h * H2 : (h + 1) * H2],
                        start=(k % G == 0),
                        stop=(k % G == G - 1),
                        tile_position=(0, part),
                    )
                # pair complete once its odd chain (or the last chain) ends
                if k % G == G - 1 and (ch % 2 == 1 or ch == NCH - 1):
                    evac_pair(ch // 2)
        nc.sync.dma_start(eout[:], e_all[:])
    nc.finalize()
    return nc


def _choose_layout(tiles_per_batch):
    """Pick chain granularity G and per-core chain count NCH.

    PSUM banks are recycled (pool of 8 rotating pair tiles), so NCH is
    only limited by keeping the nout output reasonable."""
    best = None
    for G in (1, 2, 4, 8, 16, 32, 64):
        chains_total = sum((tb + G - 1) // G for tb in tiles_per_batch)
        nch = max(1, (chains_total + NCORES - 1) // NCORES)
        if nch > MAX_NCH:
            continue
        T = nch * G
        if best is None or T < best[2]:
            best = (G, nch, T)
    if best is None:
        raise ValueError(f"no feasible layout for {tiles_per_batch}")
    return best


def kernel(hidden_states, query_indices, attention_mask, w):
    hidden_states = np.asarray(hidden_states, dtype=np.float32)
    qi = np.asarray(query_indices)
    mask = np.asarray(attention_mask, dtype=np.float32)
    w = np.asarray(w, dtype=np.float32)
    B, S, H = hidden_states.shape

    start = qi[:, 0].astype(np.int64)
    end = qi[:, 1].astype(np.int64)
    qlen = end - start
    max_len = int((end + 1 - start).max())

    mode = os.environ.get("ATTNAGG_MODE", "gather")
    if mode == "full":
        tiles_per_batch = [(S + P - 1) // P] * B
    else:
        tiles_per_batch = [int((q + P - 1) // P) for q in qlen]

    G, NCH, T = _choose_layout(tiles_per_batch)

    # ---- global chain list: (batch, first_tile_in_batch, ntiles) ----
    chains = []
    for b in range(B):
        tb = tiles_per_batch[b]
        for c in range((tb + G - 1) // G):
            chains.append((b, c * G, min(G, tb - c * G)))
    assert len(chains) <= NCORES * NCH

    # ---- pack per-core inputs ----
    wrep = np.ascontiguousarray(np.broadcast_to(w, (P, H)))
    in_maps = []
    core_chains = [[] for _ in range(NCORES)]  # (slot, batch, first_tile, ntiles)
    for i, (b, ft, nt) in enumerate(chains):
        core_chains[i // NCH].append((i % NCH, b, ft, nt))

    for c in range(NCORES):
        hs_shard = np.zeros((T, P, H), dtype=np.float32)
        aux = np.zeros((P, 2 * T), dtype=np.float32)
        aux[:, T:] = NEG_BIAS
        for slot, b, ft, nt in core_chains[c]:
            for tg in range(nt):
                k = slot * G + tg
                tb_idx = ft + tg
                if mode == "full":
                    r0 = tb_idx * P
                    r1 = min(r0 + P, S)
                    n = r1 - r0
                    hs_shard[k, :n] = hidden_states[b, r0:r1]
                    rows = np.arange(r0, r1)
                    valid = (rows >= start[b]) & (rows < end[b])
                    aux[:n, T + k] = np.where(valid, 0.0, NEG_BIAS)
                else:
                    r0 = int(start[b]) + tb_idx * P
                    r1 = min(r0 + P, int(end[b]))
                    n = r1 - r0
                    hs_shard[k, :n] = hidden_states[b, r0:r1]
                    aux[:n, T + k] = 0.0
                aux[:, k] = mask[b]
        in_maps.append({"hs": hs_shard, "wrep": wrep, "aux": aux})

    # ---- build + run ----
    key = (T, G, NCH, H)
    if key not in _NC_CACHE:
        _NC_CACHE[key] = _build_nc(T, G, NCH, H)
    nc = _NC_CACHE[key]

    trace = bool(int(os.environ.get("ATTNAGG_TRACE", "0")))
    kw = {}
    if trace:
        import concourse.bass_utils as _bu

        _bu.upload_artifacts = lambda tmpdir: "local://" + tmpdir
        tdir = os.environ.get("ATTNAGG_TRACE_DIR") or None
        if tdir:
            import glob as _glob

            for f in _glob.glob(os.path.join(tdir, "*")):
                try:
                    os.remove(f)
                except OSError:
                    pass
        kw = {"trace": True, "tmpdir": tdir}
    res = run_bass_kernel_spmd(nc, in_maps, list(range(NCORES)), **kw)
    if trace:
        kernel.last_exec_time_ns = res.exec_time_ns
        kernel.last_trace = res.instructions_and_trace

    # ---- host epilogue (tiny): combine per-chain partials ----
    num = np.zeros((B, H), dtype=np.float64)
    den = np.zeros((B,), dtype=np.float64)
    for c in range(NCORES):
        eo = res.results[c]["eout"].astype(np.float64)
        no = res.results[c]["nout"].astype(np.float64)
        for slot, b, ft, nt in core_chains[c]:
            num[b, :512] += no[2 * slot]
            num[b, 512:] += no[2 * slot + 1]
            den[b] += eo[:, slot * G : slot * G + nt].sum()
    den += (max_len - qlen).astype(np.float64)
    out = mask.astype(np.float64)[:, None] * num / den[:, None]
    return out.astype(np.float32)
